# revision 25
# baseline (speedup 1.0000x reference)
"""Trainium2 Bass kernel for nn_Embedding_loss (masked per-instance embedding loss).

Math: for each instance k with class c_k, over the (H,W) plane:
    cnt_k = sum(mask_k), s1_k = sum(emb[c_k] * mask_k), s2_k = sum(emb[c_k]^2 * mask_k)
With m1 = emb * mask and mask in {0,1}:  s2_k = sum(m1^2).
Per-instance means/variances plus the tiny O(K^2) pairwise hinge term are
assembled on the host from the (s1, s2, cnt) triples.

Sharding: K instances are split across 8 cores (13 per core, zero-padded).

Device decomposition (per core) — avoids the two 1x-mode walls (fp8 elementwise
and accum_out ops both run at 1 elem/cycle on VectorE):
  - masks staged as 0x00/0xFF bytes; m1 = plane AND mask done on VectorE at
    uint32 granularity (4 fp8 bytes/lane/cycle, exact) in one batched op/wave.
  - squares: one batched ScalarE Square activation per wave (fp8 -> fp16).
  - per-instance sums s1, s2: TensorE DoubleRow matmuls with a sliding
    pair-one-hot stationary steering each instance's column sums into its own
    PSUM row. Two VectorE passes (one per PSUM bank) drain all s1/s2 with a
    single accumulate each. A few squares run on VectorE as fp8 STT with
    fused accumulation to balance the ScalarE and TensorE streams.
"""

import os

import numpy as np

import concourse.bass as bass
import concourse.tile as tile
from concourse import mybir
from concourse.bass_utils import run_bass_kernel_spmd

N_CORES = 8
C, H, W = 80, 512, 512
K = 100
KPC = 13  # instances per core (8*13 = 104 >= 100, padded with zero masks)
P = 128  # SBUF partitions
F = (H * W) // P  # free-dim elements per partition (2048)
FW = F // 4  # uint32 words per partition per instance (512)

WAVES = [1, 4, 4, 2, 2]  # instances per DMA wave (small tail wave)
# Plane dedup: positions sharing a class read one plane slot. Uniform per-core
# pattern [3,2,2,1,1,1,1,1,1] -> 9 slots; host assignment falls back to the
# identity mapping (13 slots) when the class multiset can't fill the pattern.
PATTERN = (3, 2, 2, 1, 1, 1, 1, 1, 1)
SLOT_OF_DEDUP = (0, 0, 0, 1, 1, 2, 2, 3, 4, 5, 6, 7, 8)
SLOT_OF_IDENT = tuple(range(KPC))
DVE_SQ = (1, 5, 9, 11)  # wave-prefix instances squared on VectorE w/ accum
M1_OFF = 196608  # fixed SBUF byte offset of the dual-dtype m1 region

_NC_CACHE = {}
LAST_RESULT = None  # BassKernelResults of the most recent run (for test harness)


def _split_sync(nc, max_w=1, max_u=1):
    """Walrus in this env accepts at most one sync wait/update per instruction;
    Tile's kernel-tail drain aggregates several. Split extras onto NoOps on the
    same engine (sequential waits on one queue are an AND, so semantics hold)."""
    ctr = 0
    for f in nc.m.functions:
        for bb in f.blocks:
            new = []
            for inst in bb.instructions:
                si = getattr(inst, "sync_info", None)
                waits = list(si.on_wait) if si is not None and si.on_wait else []
                updates = (
                    list(si.on_update) if si is not None and si.on_update else []
                )
                pre, post = [], []
                if len(waits) > max_w:
                    extra, keep = waits[:-max_w], waits[-max_w:]
                    si.on_wait = keep
                    for w in extra:
                        ctr += 1
                        nop = mybir.InstNoOp(name=f"syncsplit-w-{ctr}", ins=[], outs=[])
                        nop.engine = inst.engine
                        nop.sync_info = mybir.SyncInfo(on_wait=[w], on_update=[])
                        pre.append(nop)
                if len(updates) > max_u:
                    keep_u, extra_u = updates[:max_u], updates[max_u:]
                    si.on_update = keep_u
                    for u in extra_u:
                        ctr += 1
                        nop = mybir.InstNoOp(name=f"syncsplit-u-{ctr}", ins=[], outs=[])
                        nop.engine = inst.engine
                        nop.sync_info = mybir.SyncInfo(on_wait=[], on_update=[u])
                        post.append(nop)
                new.extend(pre)
                new.append(inst)
                new.extend(post)
            bb.instructions = new


def _build_program(slot_of):
    """One SPMD Bass program: stream plane-slot/mask waves, emit (s1, s2)."""
    if slot_of in _NC_CACHE:
        return _NC_CACHE[slot_of]
    nslot = slot_of[-1] + 1

    nc = bass.Bass()
    planes = nc.declare_dram_parameter(
        "planes", [P, nslot, FW], mybir.dt.uint32, isOutput=False
    )
    masks = nc.declare_dram_parameter(
        "masks", [P, KPC, FW], mybir.dt.uint32, isOutput=False
    )
    # sliding pair-one-hot for DoubleRow matmuls: cols 26,27 are 1.0; the
    # window sel[:, 26-2j : 52-2j] puts the ones-pair at relative cols (2j, 2j+1),
    # steering instance j's sums into PSUM row j (out partitions = 26/2 = 13).
    sel8 = nc.declare_dram_parameter(
        "sel8", [P, 2, 32], mybir.dt.float8e4, isOutput=False
    )
    # stats: col 0 = s1 (rows 0..KPC = instances), col 1 = s2 (PE instances),
    # cols 2.. = per-partition s2 partials for the DVE_SQ instances
    stats = nc.declare_dram_parameter("stats", [P, 8], mybir.dt.float32, isOutput=True)

    waves = []
    lo = 0
    for w in WAVES:
        waves.append((lo, lo + w))
        lo += w
    assert lo == KPC
    # slot_of is nondecreasing; plane wave w loads slots [shi(w-1), slot_of(hi-1)+1)
    pwaves, slo = [], 0
    for lo, hi in waves:
        shi = slot_of[hi - 1] + 1
        pwaves.append((slo, shi))
        slo = shi

    # m1 region, aliased as uint32 (bitwise-AND dest) and fp8 (Square/matmul
    # src). Split into per-wave tensor pairs so the alias tracker never sees a
    # cross-wave conflict (a single aliased pair serialized AND_w behind wave
    # w-1's Square/matmul readers).
    m1u_w, m1f_w = {}, {}
    for w, (lo, hi) in enumerate(waves):
        off = M1_OFF + lo * F
        m1u_w[lo] = nc.alloc_sbuf_tensor_at(
            f"m1u{w}", [P, hi - lo, FW], mybir.dt.uint32, offset=off
        )
        m1f_w[lo] = nc.alloc_sbuf_tensor_at(
            f"m1f{w}", [P, hi - lo, 2, F // 2], mybir.dt.float8e4, offset=off
        )

    with tile.TileContext(nc) as tc:
        with (
            tc.tile_pool(name="io", bufs=4) as io,
            tc.tile_pool(name="sqp", bufs=1) as sqp,
            tc.tile_pool(name="onesp", bufs=1) as onesp,
            tc.tile_pool(name="junkp", bufs=2) as junkp,
            tc.tile_pool(name="statp", bufs=1) as statp,
            tc.tile_pool(name="ps", bufs=1, space="PSUM") as ps,
        ):
            o8 = onesp.tile([P, 2, 32], mybir.dt.float8e4, tag="o8")

            st = statp.tile([P, 8], mybir.dt.float32)
            sq = sqp.tile([P, KPC, 2, F // 2], mybir.dt.float8e4)
            ps1 = ps.tile([KPC, FW], mybir.dt.float32, tag="ps1")
            ps2 = ps.tile([KPC, FW], mybir.dt.float32, tag="ps2")

            pslots = sqp.tile([P, nslot, FW], mybir.dt.uint32, tag="pslots")
            # DMA waits are CUMULATIVE per HWDGE ring (completion order among
            # in-flight transfers is not guaranteed, so a consumer waits all
            # transfers issued on the ring before its emission point). Emit
            # masks w0+w1 up front (wave-1 then waits only 1.25MB); emit later
            # mask DMAs right after earlier AND blocks so their thresholds
            # exclude them for early waves. 8 DMAs total.
            def _mask_dma(dlo, dhi, mts):
                mt = io.tile([P, 4, FW], mybir.dt.uint32, tag="m")
                nc.scalar.dma_start(out=mt[:, : dhi - dlo, :], in_=masks[:, dlo:dhi, :])
                for j in range(dlo, dhi):
                    mts[j] = (mt, j - dlo)

            DEFERRED_MASK = {1: (5, 9), 5: (9, 13)}  # emit after this wave's ANDs
            mts = {}
            _mask_dma(0, 1, mts)
            _mask_dma(1, 3, mts)
            _mask_dma(3, 5, mts)
            # planes: 2 transfers on the sync ring (slots for waves 0-1, rest)
            nc.sync.dma_start(out=pslots[:, 0:2, :], in_=planes[:, 0:2, :])
            nc.sync.dma_start(out=o8, in_=sel8[:, :, :])
            nc.sync.dma_start(out=pslots[:, 2:nslot, :], in_=planes[:, 2:nslot, :])
            for lo, hi in waves:
                n = hi - lo

                # masked-plane per position: m1 = plane-slot AND maskFF (exact)
                m1u, m1f = m1u_w[lo], m1f_w[lo]
                for j in range(lo, hi):
                    mt, mi = mts[j]
                    nc.vector.tensor_tensor(
                        out=m1u[:, j - lo, :],
                        in0=pslots[:, slot_of[j], :],
                        in1=mt[:, mi, :],
                        op=mybir.AluOpType.bitwise_and,
                    )
                if lo in DEFERRED_MASK:
                    _mask_dma(*DEFERRED_MASK[lo], mts)
                # squares for this wave's DVE_SQ prefix: fp8 STT with fused
                # accumulation (s2 lands as per-partition partials)
                alo = lo
                while alo < hi and alo in DVE_SQ:
                    jk = junkp.tile([P, F], mybir.dt.float16, tag="jsq")
                    nc.vector.scalar_tensor_tensor(
                        out=jk,
                        in0=m1f[:, alo - lo, :, :],
                        scalar=1.0,
                        in1=m1f[:, alo - lo, :, :],
                        op0=mybir.AluOpType.mult,
                        op1=mybir.AluOpType.mult,
                        accum_out=st[:, 2 + DVE_SQ.index(alo) : 3 + DVE_SQ.index(alo)],
                    )
                    alo += 1
                # batched squares on ScalarE (fp8 in, fp8 out) for the rest
                nc.scalar.activation(
                    out=sq[:, alo:hi, :, :],
                    in_=m1f[:, alo - lo : hi - lo, :, :],
                    func=mybir.ActivationFunctionType.Square,
                )
                # per-instance sums into PSUM rows: DoubleRow fp8 matmuls
                # (reduction tile 2 -> rhs spans 1024 cols, out 512)
                pe_set = [j for j in range(KPC) if j not in DVE_SQ]
                for j in range(lo, hi):
                    for c in range(2):
                        nc.tensor.matmul(
                            ps1[:, :],
                            o8[:, :, 13 - j : 26 - j],
                            m1f[:, j - lo, :, c * FW : (c + 1) * FW],
                            start=(j == 0 and c == 0),
                            stop=(j == KPC - 1 and c == 1),
                            perf_mode=mybir.MatmulPerfMode.DoubleRow,
                            skip_group_check=True,
                        )
                    if j in DVE_SQ:
                        continue
                    for c in range(2):
                        nc.tensor.matmul(
                            ps2[:, :],
                            o8[:, :, 13 - j : 26 - j],
                            sq[:, j, :, c * FW : (c + 1) * FW],
                            start=(j == pe_set[0] and c == 0),
                            stop=(j == pe_set[-1] and c == 1),
                            perf_mode=mybir.MatmulPerfMode.DoubleRow,
                            skip_group_check=True,
                        )

            # drain: one accumulate per PSUM bank recovers all KPC sums
            j1 = junkp.tile([KPC, FW], mybir.dt.float32, tag="j1")
            j2 = junkp.tile([KPC, FW], mybir.dt.float32, tag="j2")
            nc.vector.tensor_scalar(
                out=j1,
                in0=ps1,
                scalar1=1.0,
                scalar2=0.0,
                op0=mybir.AluOpType.mult,
                op1=mybir.AluOpType.add,
                accum_out=st[0:KPC, 0:1],
            )
            nc.vector.tensor_scalar(
                out=j2,
                in0=ps2,
                scalar1=1.0,
                scalar2=0.0,
                op0=mybir.AluOpType.mult,
                op1=mybir.AluOpType.add,
                accum_out=st[0:KPC, 1:2],
            )

            nc.sync.dma_start(out=stats[:, :], in_=st)

    _NC_CACHE[slot_of] = nc
    return nc


def _enable_jax_compile_cache():
    try:
        import jax

        jax.config.update("jax_compilation_cache_dir", "/tmp/jax_neff_cache")
        jax.config.update("jax_persistent_cache_min_entry_size_bytes", -1)
        jax.config.update("jax_persistent_cache_min_compile_time_secs", 0.0)
    except Exception:
        pass
    # NEFF disk cache keyed on BIR bytes (deterministic serialization):
    # skip walrus recompiles across processes.
    try:
        import hashlib
        import shutil

        from concourse import bass2jax

        orig = bass2jax.compile_bir_kernel
        if getattr(orig, "_neff_cache_wrapped", False):
            return

        def cached_compile(bir_json, tmpdir, neff_name="file.neff"):
            h = hashlib.sha256(
                bir_json if isinstance(bir_json, bytes) else bir_json.encode()
            ).hexdigest()
            cpath = f"/tmp/neff_cache/{h}.neff"
            if os.path.exists(cpath):
                dst = os.path.join(tmpdir, neff_name)
                shutil.copy(cpath, dst)
                return dst
            out = orig(bir_json, tmpdir, neff_name=neff_name)
            os.makedirs("/tmp/neff_cache", exist_ok=True)
            shutil.copy(out, cpath)
            return out

        cached_compile._neff_cache_wrapped = True
        bass2jax.compile_bir_kernel = cached_compile
    except Exception:
        pass


def _assign(cls, k):
    """Pack instances into 8 cores x PATTERN groups (same-class groups share a
    plane slot). Returns (inst_of[8,13] with -1 pads, slot_cls[8,nslot]) or None."""
    from collections import defaultdict

    pool = defaultdict(list)
    for i in range(k):
        pool[int(cls[i])].append(i)
    remaining = {c: list(v) for c, v in pool.items()}
    inst_of = -np.ones((N_CORES, KPC), dtype=np.int64)
    nslot = len(PATTERN)
    slot_cls = np.zeros((N_CORES, nslot), dtype=np.int64)
    pos0 = np.cumsum((0,) + PATTERN)
    for s, size in enumerate(PATTERN):
        for core in range(N_CORES):
            best = None
            for c, v in remaining.items():
                if len(v) >= size and (best is None or len(v) > len(remaining[best])):
                    best = c
            if best is None:
                if size > 1:
                    return None  # pattern infeasible for this class multiset
                continue  # leftover single positions become pads
            slot_cls[core, s] = best
            for t in range(size):
                inst_of[core, pos0[s] + t] = remaining[best].pop()
            if not remaining[best]:
                del remaining[best]
    if any(remaining.values()):
        return None
    return inst_of, slot_cls


def kernel(pred_emb, gt_objmask, gt_classes):
    global LAST_RESULT
    pred_emb = np.asarray(pred_emb)
    gt_objmask = np.asarray(gt_objmask)
    cls = np.clip(np.asarray(gt_classes).astype(np.int64), 0, C - 1)
    k = gt_objmask.shape[0]

    _enable_jax_compile_cache()
    asg = _assign(cls, k) if k == N_CORES * KPC - 4 else None
    if asg is not None:
        inst_of, slot_cls = asg
        slot_of = SLOT_OF_DEDUP
    else:
        inst_of = -np.ones((N_CORES, KPC), dtype=np.int64)
        for c in range(N_CORES):
            lo, hi = c * KPC, min((c + 1) * KPC, k)
            inst_of[c, : hi - lo] = np.arange(lo, hi)
        slot_cls = np.where(inst_of >= 0, cls[np.maximum(inst_of, 0)], 0)
        slot_of = SLOT_OF_IDENT
    nslot = slot_of[-1] + 1
    nc = _build_program(slot_of)
    if not getattr(nc, "_sync_split_done", False):
        _split_sync(nc)  # CoreSim can't execute the bare NoOps; HW path only
        nc._sync_split_done = True

    f8 = mybir.dt.np(mybir.dt.float8e4)
    emb8 = pred_emb.astype(f8).reshape(C, P, F)
    maskff = (gt_objmask.astype(np.uint8) * np.uint8(0xFF)).reshape(k, P, F)
    cnt = np.count_nonzero(gt_objmask.reshape(k, -1), axis=1).astype(np.float64)

    sel8 = np.zeros((P, 2, 32), dtype=f8)
    sel8[:, :, 13] = 1.0

    in_maps = []
    for c in range(N_CORES):
        pl = emb8[slot_cls[c, :nslot]].transpose(1, 0, 2).copy().view(np.uint8)
        mk = np.zeros((P, KPC, F), dtype=np.uint8)
        for pos in range(KPC):
            iid = inst_of[c, pos]
            if iid >= 0:
                mk[:, pos] = maskff[iid]
        in_maps.append(
            {
                "planes": np.ascontiguousarray(pl).view(np.uint32),
                "masks": mk.view(np.uint32),
                "sel8": sel8,
            }
        )

    core_ids = list(range(N_CORES))
    trace = bool(os.environ.get("KERNEL_TRACE"))
    res = run_bass_kernel_spmd(
        nc,
        in_maps,
        core_ids,
        trace=trace,
        trace_cores=core_ids if trace else None,
    )
    LAST_RESULT = res

    s1 = np.zeros(k, dtype=np.float64)
    s2 = np.zeros(k, dtype=np.float64)
    for c in range(N_CORES):
        stats = res.results[c]["stats"].astype(np.float64)  # (P, 8)
        for pos in range(KPC):
            iid = inst_of[c, pos]
            if iid < 0:
                continue
            s1[iid] = stats[pos, 0]
            if pos in DVE_SQ:
                s2[iid] = stats[:, 2 + DVE_SQ.index(pos)].sum()
            else:
                s2[iid] = stats[pos, 1]

    has = cnt > 0
    safe = np.where(has, cnt, 1.0)
    mean = np.where(has, s1 / safe, 0.0)
    var = np.where(has, s2 / safe - mean * mean, 0.0)

    same = cls[:, None] == cls[None, :]
    upper = np.triu(np.ones((k, k), dtype=bool), 1)
    diff2 = (mean[:, None] - mean[None, :]) ** 2
    hinge = np.maximum(1.0 - diff2, 0.0)
    loss_inter = np.sum(np.where(same & upper, hinge, 0.0))
    loss_reg = np.mean(mean * mean)
    loss_intra = np.mean(var)
    loss = 1.0 * loss_inter + 1.0 * loss_reg + 1.0 * loss_intra
    return np.array([loss], dtype=np.float32)
